# revision 5
# baseline (speedup 1.0000x reference)
"""Trainium2 Bass kernel for nn_Criterion_28278064676991.

Loss:
  a = batch[anchors]
  d_pos = ||a_i - b_{pos_idx[i,p]} + eps||_2     (P=8)
  d_neg = ||a_i - b_{neg_idx[i,q]} + eps||_2     (Q=32)
  per_anchor = relu(logsumexp(d_pos) + logsumexp(1 - d_neg))
  loss = mean(per_anchor) + 0.005 * mean(||b_j||)

Strategy (8-way data parallel over anchors, 512 anchors/core):
  d^2(i,j) = |a_i|^2 + |b_j|^2 - 2 a_i.b_j + 2e(s_ai - s_bj) + D e^2
  The only superlinear term is the cross Gram matrix G[i,j] = -2 a_i.b_j,
  computed on the PE as (-2 A_c)^T-stationary x B^T-moving.  All
  j-dependent scalar corrections (|b_j|^2 - 2e s_bj at the selected
  indices) and the per-anchor terms are O(N) and pre-gathered on the host
  (indices are kernel inputs, not device-computed).  G goes through an
  internal-DRAM round trip so the per-anchor scalar gather can use the
  GPSIMD indirect DMA (element-granular descriptors); the logsumexp tail
  then runs on [128, 40] tiles.  Per-core partial sums are combined on the
  host (scalar output).
"""

import numpy as np

import concourse.bass as bass
import concourse.mybir as mybir
import concourse.tile as tile_mod
from concourse.bass_utils import run_bass_kernel_spmd
from concourse.tile import TileContext

N, D, P, Q = 4096, 512, 8, 32
T = P + Q  # 40 selected neighbors per anchor
MARGIN = 1.0
L2_WEIGHT = 0.005
EPS = 1e-6
N_CORES = 8
NL = N // N_CORES  # 512 local anchors per core
N_IT = NL // 128  # 4 partition tiles per core
NJ = N // 512  # 8 matmul free-dim blocks
NK = D // 128  # 4 contraction tiles

F32 = mybir.dt.float32
MM_DT = mybir.dt.float32  # Gram matmul input dtype
RT_DT = mybir.dt.float32  # G DRAM round-trip dtype

AF = mybir.ActivationFunctionType
ALU = mybir.AluOpType
AX = mybir.AxisListType


def _split_multi_waits(nc):
    """walrus in this container encodes at most one sync-wait per
    instruction; hoist extra waits onto same-engine NoOps inserted just
    before the over-limit instruction (sequencer-order equivalent)."""
    counter = [0]
    for fn in nc.m.functions:
        for bb in fn.blocks:
            new_insts = []
            changed = False
            for inst in bb.instructions:
                si = inst.sync_info
                if si is not None and si.on_wait is not None and len(si.on_wait) > 1:
                    waits = list(si.on_wait)
                    for w in waits[:-1]:
                        nop = mybir.InstNoOp(
                            name=f"waitspill-{counter[0]}", ins=[], outs=[]
                        )
                        counter[0] += 1
                        nop.engine = inst.engine
                        nop.sync_info = mybir.SyncInfo(on_wait=[w], on_update=[])
                        new_insts.append(nop)
                    inst.sync_info = mybir.SyncInfo(
                        on_wait=[waits[-1]], on_update=list(si.on_update or [])
                    )
                    changed = True
                new_insts.append(inst)
            if changed:
                bb.instructions = new_insts


def build_nc(split_waits=True):
    nc = bass.Bass()
    bhat = nc.dram_tensor("bhat", [D, N], MM_DT, kind="ExternalInput")
    ahat = nc.dram_tensor("ahat", [D, NL], MM_DT, kind="ExternalInput")
    aadd = nc.dram_tensor("aadd", [128, N_IT], F32, kind="ExternalInput")
    cterm = nc.dram_tensor("cterm", [128, N_IT * T], F32, kind="ExternalInput")
    goff = nc.dram_tensor("goff", [128, N_IT * T], mybir.dt.int32, kind="ExternalInput")
    n2 = nc.dram_tensor("n2", [128, N // 128], F32, kind="ExternalInput")
    out = nc.dram_tensor("out", [1, 2], F32, kind="ExternalOutput")
    gdram = nc.dram_tensor("gscratch", [NL, N], RT_DT)

    with TileContext(nc) as tc:
        with (
            tc.tile_pool(name="big", bufs=1) as big,
            tc.tile_pool(name="gpool", bufs=2) as gpool,
            tc.tile_pool(name="small", bufs=24) as small,
            tc.tile_pool(name="ps", bufs=8, space="PSUM") as ps,
        ):
            bhat_sb = []
            ahat_sb = []
            for k in range(NK):
                bt = big.tile([128, N], MM_DT, tag=f"bh{k}")
                nc.sync.dma_start(out=bt[:], in_=bhat[128 * k : 128 * (k + 1), :])
                bhat_sb.append(bt)
                at = big.tile([128, NL], MM_DT, tag=f"ah{k}")
                nc.sync.dma_start(out=at[:], in_=ahat[128 * k : 128 * (k + 1), :])
                ahat_sb.append(at)
            aadd_sb = big.tile([128, N_IT], F32, tag="aadd")
            nc.sync.dma_start(out=aadd_sb[:], in_=aadd[:])
            ct_sb = big.tile([128, N_IT * T], F32, tag="ct")
            nc.sync.dma_start(out=ct_sb[:], in_=cterm[:])
            go_sb = big.tile([128, N_IT * T], mybir.dt.int32, tag="go")
            nc.sync.dma_start(out=go_sb[:], in_=goff[:])
            n2_sb = big.tile([128, N // 128], F32, tag="n2")
            nc.sync.dma_start(out=n2_sb[:], in_=n2[:])
            ones_sb = big.tile([128, 1], F32, tag="ones")
            nc.vector.memset(ones_sb[:], 1.0)

            pa_sb = big.tile([128, N_IT], F32, tag="pa")

            for it in range(N_IT):
                ptiles = [
                    ps.tile([128, 512], F32, tag="ps", name=f"pt{it}_{j}")
                    for j in range(NJ)
                ]
                for k in range(NK):
                    lhsT = ahat_sb[k][:, 128 * it : 128 * (it + 1)]
                    for j in range(NJ):
                        nc.tensor.matmul(
                            ptiles[j][:],
                            lhsT=lhsT,
                            rhs=bhat_sb[k][:, 512 * j : 512 * (j + 1)],
                            start=(k == 0),
                            stop=(k == NK - 1),
                        )
                g_sb = gpool.tile([128, N], RT_DT, tag="g")
                for j in range(NJ):
                    src = ptiles[j][:]
                    dst = g_sb[:, 512 * j : 512 * (j + 1)]
                    if j % 2 == 0:
                        nc.vector.tensor_copy(out=dst, in_=src)
                    else:
                        nc.scalar.copy(out=dst, in_=src)
                wr = nc.sync.dma_start(
                    out=gdram[128 * it : 128 * (it + 1), :], in_=g_sb[:]
                )
                sel_sb = small.tile([128, T], RT_DT, tag="sel")
                rd = nc.gpsimd.indirect_dma_start(
                    out=sel_sb[:],
                    out_offset=None,
                    in_=gdram[:],
                    in_offset=bass.IndirectOffsetOnAxis(
                        ap=go_sb[:, T * it : T * (it + 1)], axis=1
                    ),
                )
                tile_mod.add_dep_helper(
                    rd.ins, wr.ins, reason="indirect gather reads gscratch i-tile"
                )

                # d = sqrt(G_sel + cterm_sel + aadd_i)
                d2 = small.tile([128, T], F32, tag="d2")
                nc.vector.tensor_add(
                    out=d2[:], in0=sel_sb[:], in1=ct_sb[:, T * it : T * (it + 1)]
                )
                d2c = small.tile([128, T], F32, tag="d2c")
                nc.vector.tensor_scalar(
                    out=d2c[:],
                    in0=d2[:],
                    scalar1=aadd_sb[:, it : it + 1],
                    scalar2=0.0,
                    op0=ALU.add,
                    op1=ALU.max,
                )
                d_sb = small.tile([128, T], F32, tag="d")
                nc.scalar.activation(out=d_sb[:], in_=d2c[:], func=AF.Sqrt)
                # pos_term = logsumexp(d[:, :P])
                mp = small.tile([128, 1], F32, tag="mp")
                nc.vector.tensor_reduce(
                    out=mp[:], in_=d_sb[:, 0:P], axis=AX.X, op=ALU.max
                )
                nmp = small.tile([128, 1], F32, tag="nmp")
                nc.vector.tensor_scalar_mul(out=nmp[:], in0=mp[:], scalar1=-1.0)
                ep = small.tile([128, P], F32, tag="ep")
                sp = small.tile([128, 1], F32, tag="sp")
                nc.scalar.activation(
                    out=ep[:],
                    in_=d_sb[:, 0:P],
                    func=AF.Exp,
                    bias=nmp[:],
                    scale=1.0,
                    accum_out=sp[:],
                )
                lp = small.tile([128, 1], F32, tag="lp")
                nc.scalar.activation(out=lp[:], in_=sp[:], func=AF.Ln)
                # neg_term = logsumexp(MARGIN - d[:, P:]) = MARGIN - mn + ln(sum)
                # with mn = min(d_neg):  exp((mn - d)) summed
                mn = small.tile([128, 1], F32, tag="mn")
                nc.vector.tensor_reduce(
                    out=mn[:], in_=d_sb[:, P:T], axis=AX.X, op=ALU.min
                )
                en = small.tile([128, Q], F32, tag="en")
                sn = small.tile([128, 1], F32, tag="sn")
                nc.scalar.activation(
                    out=en[:],
                    in_=d_sb[:, P:T],
                    func=AF.Exp,
                    bias=mn[:],
                    scale=-1.0,
                    accum_out=sn[:],
                )
                lsn = small.tile([128, 1], F32, tag="lsn")
                nc.scalar.activation(out=lsn[:], in_=sn[:], func=AF.Ln)
                # per_anchor = relu(lp + mp + lsn - mn + MARGIN)
                t1 = small.tile([128, 1], F32, tag="t1")
                nc.vector.tensor_add(out=t1[:], in0=lp[:], in1=mp[:])
                t2 = small.tile([128, 1], F32, tag="t2")
                nc.vector.tensor_add(out=t2[:], in0=t1[:], in1=lsn[:])
                t3 = small.tile([128, 1], F32, tag="t3")
                nc.vector.tensor_sub(out=t3[:], in0=t2[:], in1=mn[:])
                nc.scalar.activation(
                    out=pa_sb[:, it : it + 1],
                    in_=t3[:],
                    func=AF.Relu,
                    bias=MARGIN,
                    scale=1.0,
                )

            # l2 partial: sum over all partitions/cols of sqrt(n2)
            sq = small.tile([128, N // 128], F32, tag="sq")
            l2p = small.tile([128, 1], F32, tag="l2p")
            nc.scalar.activation(
                out=sq[:], in_=n2_sb[:], func=AF.Sqrt, accum_out=l2p[:]
            )
            # partition-reduce via ones matmul
            pfin = ps.tile([1, 8], F32, tag="ps")
            nc.tensor.matmul(
                pfin[:, 0:N_IT], lhsT=ones_sb[:], rhs=pa_sb[:], start=True, stop=True
            )
            nc.tensor.matmul(
                pfin[:, N_IT : N_IT + 1],
                lhsT=ones_sb[:],
                rhs=l2p[:],
                start=True,
                stop=True,
            )
            res_sb = small.tile([1, 2], F32, tag="res")
            nc.vector.tensor_reduce(
                out=res_sb[:, 0:1], in_=pfin[:1, 0:N_IT], axis=AX.X, op=ALU.add
            )
            nc.vector.tensor_copy(
                out=res_sb[:, 1:2], in_=pfin[:1, N_IT : N_IT + 1]
            )
            nc.sync.dma_start(out=out[:], in_=res_sb[:])
    if split_waits:
        _split_multi_waits(nc)
    return nc


_NC_CACHE = None


def _get_nc():
    global _NC_CACHE
    if _NC_CACHE is None:
        _NC_CACHE = build_nc()
    return _NC_CACHE


def _np_dt(dt):
    return {F32: np.float32, mybir.dt.bfloat16: "bfloat16"}[dt]


def host_prep(batch, anchors, pos_idx, neg_idx):
    """Build the per-core input maps (O(N*D) host work: layout + norms)."""
    import ml_dtypes  # noqa: F401  (registers bfloat16 with numpy)

    batch = np.asarray(batch, dtype=np.float32)
    anchors = np.asarray(anchors).astype(np.int64)
    pos_idx = np.asarray(pos_idx).astype(np.int64)
    neg_idx = np.asarray(neg_idx).astype(np.int64)

    b64 = batch.astype(np.float64)
    norms2 = np.einsum("ij,ij->i", b64, b64)
    ssum = b64.sum(axis=1)

    aadd_all = (norms2[anchors] + 2.0 * EPS * ssum[anchors] + D * EPS * EPS).astype(
        np.float32
    )
    sel = np.concatenate([pos_idx, neg_idx], axis=1)  # [N, T]
    cterm_all = (norms2[sel] - 2.0 * EPS * ssum[sel]).astype(np.float32)

    mm_np = np.dtype(_np_dt(MM_DT))
    bhat = np.ascontiguousarray(batch.T).astype(mm_np)  # [D, N]
    a_all = batch[anchors]  # [N, D]

    n2_c0 = np.ascontiguousarray(
        norms2.astype(np.float32).reshape(N // 128, 128).T
    )  # [128, N/128]
    n2_zero = np.zeros_like(n2_c0)

    in_maps = []
    for c in range(N_CORES):
        r0, r1 = c * NL, (c + 1) * NL
        ahat_c = np.ascontiguousarray((-2.0 * a_all[r0:r1]).T).astype(mm_np)
        aadd_c = np.ascontiguousarray(aadd_all[r0:r1].reshape(N_IT, 128).T)
        ct_c = np.ascontiguousarray(
            cterm_all[r0:r1].reshape(N_IT, 128, T).transpose(1, 0, 2).reshape(128, -1)
        )
        li = np.arange(NL, dtype=np.int64)
        gf = (li[:, None] * N + sel[r0:r1]).astype(np.int32)  # [NL, T]
        go_c = np.ascontiguousarray(
            gf.reshape(N_IT, 128, T).transpose(1, 0, 2).reshape(128, -1)
        )
        in_maps.append(
            {
                "bhat": bhat,
                "ahat": ahat_c,
                "aadd": aadd_c,
                "cterm": ct_c,
                "goff": go_c,
                "n2": n2_c0 if c == 0 else n2_zero,
            }
        )
    return in_maps


def combine(results):
    pa_sum = sum(float(r["out"][0, 0]) for r in results)
    l2_sum = sum(float(r["out"][0, 1]) for r in results)
    loss = pa_sum / N + L2_WEIGHT * l2_sum / N
    return np.asarray(loss, dtype=np.float32)


def kernel(batch, anchors, pos_idx, neg_idx):
    nc = _get_nc()
    in_maps = host_prep(batch, anchors, pos_idx, neg_idx)
    res = run_bass_kernel_spmd(nc, in_maps, list(range(N_CORES)))
    return combine(res.results)
